# revision 3
# baseline (speedup 1.0000x reference)
"""Trainium2 Bass/Tile kernel for nn_Attention_30511447671564.

kernel(**inputs) takes FULL inputs, shards batch B=64 across 8 NeuronCores
(data-parallel, zero collectives), runs a Bass/Tile kernel per core, and
gathers the full (64, 64, 4096) output.

Per-core program (512 tokens):
  stage 1  xT = x^T via PE transposes                   (bf16)
  stage 2  GEMM1 qkvT[o,t] = W_atten @ x^T + b          (W streamed fp32->bf16
           via SWDGE cast-DMA, transposed on the DMA XBAR; per-partition bias
           + 1/sqrt(hd) k-scaling fused into the ACT evacuation)
  stage 3  per-token HxH head-mixing attention: attT = k.q^T (half-array
           matmul), +mask, exp on ACT, y = exp^T @ [v|1] (fused softmax
           denominator via ones column), PE transposes back to yT[c,t]
  stage 4  GEMM2 out[t,o] = y @ W_proj^T + b (bias via K=1 ones-row matmul)
"""
import numpy as np

import concourse.bacc as bacc
import concourse.tile as tile
from concourse import mybir

F32 = mybir.dt.float32
BF16 = mybir.dt.bfloat16
AFT = mybir.ActivationFunctionType

TOK = 512
C = 4096
O3 = 3 * C
H = 64
KT = C // 128          # 32
NOT = O3 // 128        # 96
TW = 128               # attention window tokens
OSL = 256              # GEMM1 o-slice
NSL = O3 // OSL        # 48
OSL2 = 512             # GEMM2 o-slice
NSL2 = C // OSL2       # 8
TB = 4                 # attention token batch
NEG = -1.0e30


def build_nc():
    nc = bacc.Bacc("TRN2", target_bir_lowering=False, debug=False, num_devices=8)

    x = nc.dram_tensor("x", [TOK, C], F32, kind="ExternalInput").ap()
    wa = nc.dram_tensor("W_atten", [O3, C], F32, kind="ExternalInput").ap()
    ba = nc.dram_tensor("b_atten", [O3], F32, kind="ExternalInput").ap()
    wp = nc.dram_tensor("W_proj", [C, C], F32, kind="ExternalInput").ap()
    bp = nc.dram_tensor("b_proj", [C], F32, kind="ExternalInput").ap()
    idm = nc.dram_tensor("idmat", [128, 128], F32, kind="ExternalInput").ap()
    msk = nc.dram_tensor("maskT", [64, 64], F32, kind="ExternalInput").ap()
    out = nc.dram_tensor("out", [TOK, C], F32, kind="ExternalOutput").ap()

    with tile.TileContext(nc) as tc:
        const_p = tc.alloc_tile_pool(name="const", bufs=1)
        ident32 = const_p.tile([128, 128], F32)
        nc.sync.dma_start(ident32[:], idm[:])
        ident = const_p.tile([128, 128], BF16)
        nc.vector.tensor_copy(ident[:], ident32[:])
        maskT4 = const_p.tile([64, TB, 64], F32)
        for b in range(TB):
            nc.sync.dma_start(maskT4[:, b, :], msk[:])
        bias_a = const_p.tile([128, NOT], F32)
        nc.sync.dma_start(bias_a[:], ba.rearrange("(j p) -> p j", p=128))
        # fold 1/sqrt(HD) into the k-part biases (k rows = o-tiles 32..63)
        nc.vector.tensor_scalar_mul(bias_a[:, 32:64], bias_a[:, 32:64], 0.125)
        bias_p = const_p.tile([1, C], BF16)
        nc.gpsimd.dma_start(bias_p[:], bp[None, :])
        ones_row = const_p.tile([1, 128], BF16)
        nc.vector.memset(ones_row[:], 1.0)

        xy_p = tc.alloc_tile_pool(name="xy", bufs=1)
        qkvT_p = tc.alloc_tile_pool(name="qkvT", bufs=1)
        xT = xy_p.tile([128, KT, TOK], BF16, tag="xy")
        qkvT = qkvT_p.tile([128, NOT, TOK], BF16)

        # ---------- stage 1: xT = x^T (PE transpose, f32 in -> bf16 out) ----
        with tc.tile_pool(name="xnat", bufs=2) as xnat_p, \
             tc.tile_pool(name="ps_x", bufs=3, space="PSUM") as psx_p:
            for tt in range(TOK // 128):
                xn = xnat_p.tile([128, C], F32)
                nc.sync.dma_start(xn[:], x[tt * 128:(tt + 1) * 128, :])
                for kt in range(KT):
                    ps = psx_p.tile([128, 128], F32)
                    nc.tensor.transpose(ps[:], xn[:, kt * 128:(kt + 1) * 128],
                                        ident32[:])
                    nc.vector.tensor_copy(xT[:, kt, tt * 128:(tt + 1) * 128],
                                          ps[:])

        # ---------- stage 2: GEMM1 + W_atten^T streaming ----------
        with tc.tile_pool(name="wnat", bufs=2) as wnat_p, \
             tc.tile_pool(name="wT", bufs=3) as wT_p, \
             tc.tile_pool(name="ps_g1", bufs=4, space="PSUM") as psg1_p:
            for ot in range(NOT):
                wn = wnat_p.tile([128, C], BF16, tag="wn")
                nc.gpsimd.dma_start(
                    wn[:], wa[ot * 128:(ot + 1) * 128, :])
                wt = wT_p.tile([128, KT, 128], BF16, tag="wt")
                nc.sync.dma_start(wt[:], wn[:], transpose=True)
                ps = psg1_p.tile([128, TOK], F32)
                for kt in range(KT):
                    nc.tensor.matmul(ps[:], wt[:, kt, :], xT[:, kt, :],
                                     start=(kt == 0), stop=(kt == KT - 1))
                scale = 0.125 if 32 <= ot < 64 else 1.0
                nc.scalar.activation(qkvT[:, ot, :], ps[:], AFT.Identity,
                                     bias=bias_a[:, ot:ot + 1], scale=scale)

        # ---------- stage 3: attention ----------
        yT = xy_p.tile([128, KT, TOK], BF16, tag="xy")

        with tc.tile_pool(name="qwin", bufs=1) as qwin_p, \
             tc.tile_pool(name="kwin", bufs=1) as kwin_p, \
             tc.tile_pool(name="attbuf", bufs=3) as att_p, \
             tc.tile_pool(name="vbuf", bufs=3) as vb_p, \
             tc.tile_pool(name="ynorm", bufs=3) as yn_p, \
             tc.tile_pool(name="ps_att", bufs=2, space="PSUM") as psa_p, \
             tc.tile_pool(name="ps_v", bufs=2, space="PSUM") as psv_p, \
             tc.tile_pool(name="ps_y", bufs=2, space="PSUM") as psy_p, \
             tc.tile_pool(name="ps_yT", bufs=2, space="PSUM") as psyt_p:
            for w in range(TOK // TW):
                t0 = w * TW
                # h_pi layout: even heads at 0..31, odd heads at 32..63
                qw = qwin_p.tile([64, H, TW], BF16)
                kw = kwin_p.tile([64, H, TW], BF16)
                for par in range(2):
                    nc.sync.dma_start(
                        qw[:, par * 32:(par + 1) * 32, :],
                        qkvT[par * 64:(par + 1) * 64, 0:32, t0:t0 + TW])
                    nc.sync.dma_start(
                        kw[:, par * 32:(par + 1) * 32, :],
                        qkvT[par * 64:(par + 1) * 64, 32:64, t0:t0 + TW])

                for g in range(TW // TB):
                    att_ps = psa_p.tile([64, TB, 64], F32)
                    v_ps = psv_p.tile([64, TB, 64], BF16)
                    y_ps = psy_p.tile([64, TB, 65], F32)
                    yt_ps = psyt_p.tile([128, TB, 32], BF16)
                    attf = att_p.tile([64, TB, 64], F32)
                    expT = att_p.tile([64, TB, 64], BF16)
                    vt = vb_p.tile([64, TB, 65], BF16)
                    yn = yn_p.tile([64, TB, 64], BF16)
                    rec = yn_p.tile([64, TB], F32)
                    for b in range(TB):
                        t = g * TB + b
                        nc.tensor.matmul(att_ps[:, b, :],
                                         kw[:, :, t], qw[:, :, t],
                                         start=True, stop=True)
                        vsrc = qkvT[:, 64:96, t0 + t]
                        nc.tensor.transpose(v_ps[0:32, b, :],
                                            vsrc[0:64], ident[0:64, 0:64])
                        nc.tensor.transpose(v_ps[32:64, b, :],
                                            vsrc[64:128], ident[64:128, 64:128])
                    nc.vector.tensor_add(attf[:], att_ps[:], maskT4[:])
                    nc.scalar.activation(expT[:], attf[:], AFT.Exp)
                    nc.gpsimd.memset(vt[:, :, 64], 1.0)
                    nc.vector.tensor_copy(vt[:, :, 0:64], v_ps[:])
                    for b in range(TB):
                        nc.tensor.matmul(y_ps[:, b, :],
                                         expT[:, b, :], vt[:, b, :],
                                         start=True, stop=True)
                    nc.vector.reciprocal(rec[:], y_ps[:, :, 64])
                    for b in range(TB):
                        nc.vector.tensor_scalar_mul(yn[:, b, :],
                                                    y_ps[:, b, 0:64],
                                                    rec[:, b:b + 1])
                        nc.tensor.transpose(yt_ps[0:64, b, :],
                                            yn[0:32, b, :], ident[0:32, 0:32])
                        nc.tensor.transpose(yt_ps[64:128, b, :],
                                            yn[32:64, b, :], ident[32:64, 32:64])
                    tb0 = t0 + g * TB
                    for par in range(2):
                        nc.vector.tensor_copy(
                            yT[par * 64:(par + 1) * 64, 0:32, tb0:tb0 + TB]
                            .rearrange("d c t -> d t c"),
                            yt_ps[par * 64:(par + 1) * 64])

        qkvT_p.release()

        # ---------- stage 4: GEMM2 + W_proj^T streaming ----------
        with tc.tile_pool(name="wnat2", bufs=2) as wnat2_p, \
             tc.tile_pool(name="wT2", bufs=2) as wT2_p, \
             tc.tile_pool(name="ps_g2", bufs=4, space="PSUM") as psg2_p, \
             tc.tile_pool(name="oev", bufs=4) as oev_p:
            for s in range(NSL2):
                ni = OSL2 // 128
                wn = wnat2_p.tile([128, ni, C], BF16)
                for i in range(ni):
                    nc.gpsimd.dma_start(
                        wn[:, i, :],
                        wp[s * OSL2 + i * 128: s * OSL2 + (i + 1) * 128, :])
                wt = wT2_p.tile([128, KT, ni, 128], BF16)
                for i in range(ni):
                    nc.sync.dma_start(wt[:, :, i, :], wn[:, i, :], transpose=True)
                for tt in range(TOK // 128):
                    ps = psg2_p.tile([128, OSL2], F32)
                    for kt in range(KT):
                        nc.tensor.matmul(ps[:],
                                         yT[:, kt, tt * 128:(tt + 1) * 128],
                                         wt[:, kt, :, :],
                                         start=(kt == 0), stop=False)
                    nc.tensor.matmul(ps[:], ones_row[:],
                                     bias_p[:, s * OSL2:(s + 1) * OSL2],
                                     start=False, stop=True)
                    ev = oev_p.tile([128, OSL2], F32)
                    nc.vector.tensor_copy(ev[:], ps[:])
                    nc.sync.dma_start(
                        out[tt * 128:(tt + 1) * 128, s * OSL2:(s + 1) * OSL2],
                        ev[:])

        xy_p.release()
        const_p.release()

    nc.compile()
    return nc


def make_const_inputs():
    ident = np.eye(128, dtype=np.float32)
    pi = np.concatenate([np.arange(0, 64, 2), np.arange(1, 64, 2)])
    maskT = np.where(pi[:, None] <= pi[None, :], 0.0, NEG).astype(np.float32)
    return ident, maskT


N_CORES = 8
_CACHE = {}


def _get_nc():
    if "nc" not in _CACHE:
        _CACHE["nc"] = build_nc()
    return _CACHE["nc"]


def kernel(x, W_atten, b_atten, W_proj, b_proj):
    from concourse.bass_utils import run_bass_kernel_spmd

    x = np.ascontiguousarray(np.asarray(x, dtype=np.float32))
    W_atten = np.ascontiguousarray(np.asarray(W_atten, dtype=np.float32))
    b_atten = np.ascontiguousarray(np.asarray(b_atten, dtype=np.float32))
    W_proj = np.ascontiguousarray(np.asarray(W_proj, dtype=np.float32))
    b_proj = np.ascontiguousarray(np.asarray(b_proj, dtype=np.float32))

    nc = _get_nc()
    ident, maskT = make_const_inputs()
    B, T, Cx = x.shape
    shard = B // N_CORES
    in_maps = []
    for i in range(N_CORES):
        in_maps.append({
            "x": x[i * shard:(i + 1) * shard].reshape(TOK, C),
            "W_atten": W_atten, "b_atten": b_atten,
            "W_proj": W_proj, "b_proj": b_proj,
            "idmat": ident, "maskT": maskT,
        })
    res = run_bass_kernel_spmd(nc, in_maps, list(range(N_CORES)))
    out = np.empty((B, T, Cx), dtype=np.float32)
    for i in range(N_CORES):
        out[i * shard:(i + 1) * shard] = res.results[i]["out"].reshape(shard, T, Cx)
    return out


# revision 14
# speedup vs baseline: 24.0223x; 24.0223x over previous
"""Trainium2 Bass/Tile kernel for nn_Attention_30511447671564.

kernel(**inputs) takes FULL inputs, shards batch B=64 across 8 NeuronCores
(data-parallel, zero collectives), runs a Bass/Tile kernel per core, and
gathers the full (64, 64, 4096) output.

Per-core program (512 tokens):
  stage 1  xT = x^T via PE transposes                   (bf16)
  stage 2  GEMM1 qkvT[o,t] = W_atten @ x^T + b          (W streamed fp32->bf16
           via SWDGE cast-DMA, transposed on the DMA XBAR; per-partition bias
           + 1/sqrt(hd) k-scaling fused into the ACT evacuation)
  stage 3  per-token HxH head-mixing attention: attT = k.q^T (half-array
           matmul), +mask, exp on ACT, y = exp^T @ [v|1] (fused softmax
           denominator via ones column), PE transposes back to yT[c,t]
  stage 4  GEMM2 out[t,o] = y @ W_proj^T + b (bias via K=1 ones-row matmul)
"""
import numpy as np

import concourse.bass as bass
import concourse.bacc as bacc
import concourse.tile as tile
from concourse import mybir

F32 = mybir.dt.float32
BF16 = mybir.dt.bfloat16
AFT = mybir.ActivationFunctionType

TOK = 512
C = 4096
O3 = 3 * C
H = 64
KT = C // 128          # 32
NOT = O3 // 128        # 96
TW = 128               # attention window tokens
OSL = 256              # GEMM1 o-slice
NSL = O3 // OSL        # 48
OSL2 = 512             # GEMM2 o-slice
NSL2 = C // OSL2       # 8
TB = 4                 # attention token batch
NEG = -1.0e30
PE8 = 2                # of every 8 W o-tiles, this many go via PE-transpose


def build_nc(stages="1234", reps=1):
    nc = bacc.Bacc("TRN2", target_bir_lowering=False, debug=False, num_devices=8)

    x = nc.dram_tensor("x", [TOK, C], F32, kind="ExternalInput").ap()
    wa = nc.dram_tensor("W_atten", [O3, C], F32, kind="ExternalInput").ap()
    ba = nc.dram_tensor("b_atten", [O3], F32, kind="ExternalInput").ap()
    wp = nc.dram_tensor("W_proj", [C, C], F32, kind="ExternalInput").ap()
    bp = nc.dram_tensor("b_proj", [C], F32, kind="ExternalInput").ap()
    idm = nc.dram_tensor("idmat", [128, 128], F32, kind="ExternalInput").ap()
    msk = nc.dram_tensor("maskT", [64, 64], F32, kind="ExternalInput").ap()
    out = nc.dram_tensor("out", [TOK, C], F32, kind="ExternalOutput").ap()

    with tile.TileContext(nc) as tc:
        const_p = tc.alloc_tile_pool(name="const", bufs=1)
        ident32 = const_p.tile([128, 128], F32)
        nc.sync.dma_start(ident32[:], idm[:])
        ident = const_p.tile([128, 128], BF16)
        nc.vector.tensor_copy(ident[:], ident32[:])
        maskT4 = const_p.tile([64, TB, 64], F32)
        for b in range(TB):
            nc.sync.dma_start(maskT4[:, b, :], msk[:])
        bias_a = const_p.tile([128, NOT], F32)
        nc.sync.dma_start(bias_a[:], ba.rearrange("(j p) -> p j", p=128))
        # fold 1/sqrt(HD) into the k-part biases (k rows = o-tiles 32..63)
        nc.vector.tensor_scalar_mul(bias_a[:, 32:64], bias_a[:, 32:64], 0.125)
        bias_p = const_p.tile([1, C], BF16)
        nc.gpsimd.dma_start(bias_p[:], bp[None, :])
        ones_row = const_p.tile([1, 128], BF16)
        nc.vector.memset(ones_row[:], 1.0)

        for _rep in range(reps):
            _kernel_body(nc, tc, stages, x, wa, ba, wp, bp, out,
                         ident32, ident, maskT4, bias_a, bias_p, ones_row)
        const_p.release()

    nc.compile()
    return nc


def _kernel_body(nc, tc, stages, x, wa, ba, wp, bp, out,
                 ident32, ident, maskT4, bias_a, bias_p, ones_row):
    if True:
        xy_p = tc.alloc_tile_pool(name="xy", bufs=1)
        qkvT_p = tc.alloc_tile_pool(name="qkvT", bufs=1)
        xT = qkvT = None
        if "1" in stages or "2" in stages:
            xT = xy_p.tile([128, KT, TOK], BF16, tag="xy", name="xT")
            if "1" not in stages:
                nc.vector.memset(xT[:], 0.0)
        if "2" in stages or "3" in stages:
            qkvT = qkvT_p.tile([128, NOT, TOK], BF16, name="qkvT")
            if "2" not in stages:
                nc.vector.memset(qkvT[:], 0.0)

        # ---------- stage 1: xT = x^T (PE transpose, f32 in -> bf16 out) ----
        with tc.tile_pool(name="xnat", bufs=2) as xnat_p, \
             tc.tile_pool(name="ps_x", bufs=3, space="PSUM") as psx_p:
            for tt in range(TOK // 128 if "1" in stages else 0):
                xn = xnat_p.tile([128, C], F32)
                nc.sync.dma_start(xn[:], x[tt * 128:(tt + 1) * 128, :])
                for kt in range(KT):
                    ps = psx_p.tile([128, 128], F32)
                    nc.tensor.transpose(ps[:], xn[:, kt * 128:(kt + 1) * 128],
                                        ident32[:])
                    nc.vector.tensor_copy(xT[:, kt, tt * 128:(tt + 1) * 128],
                                          ps[:])

        # W^T production: cast-DMA HBM->SBUF bf16, then transpose either on
        # the DMA XBAR (ACT HWDGE ring) or on the PE (+ACT evacuation).
        def w_tile_T(w_dram, ot, wnat_p, wT_p, psw_p):
            wn = wnat_p.tile([128, C], BF16, tag="wn")
            nc.gpsimd.dma_start(wn[:], w_dram[ot * 128:(ot + 1) * 128, :])
            wt = wT_p.tile([128, KT, 128], BF16, tag="wt")
            if (ot % 8) < PE8:
                for k4 in range(KT // 4):
                    psw = psw_p.tile([128, 4, 128], BF16, tag="psw")
                    for j in range(4):
                        kt = k4 * 4 + j
                        nc.tensor.transpose(psw[:, j, :],
                                            wn[:, kt * 128:(kt + 1) * 128],
                                            ident[:])
                    dst = wt[:, k4 * 4:(k4 + 1) * 4, :]
                    if (ot + k4) % 2 == 0:
                        nc.scalar.activation(dst, psw[:], AFT.Copy)
                    else:
                        nc.vector.tensor_copy(dst, psw[:])
            else:
                nc.scalar.dma_start(wt[:], wn[:], transpose=True)
            return wt

        # ---------- stage 2: GEMM1 + W_atten^T streaming ----------
        with tc.tile_pool(name="wnat", bufs=2) as wnat_p, \
             tc.tile_pool(name="wT", bufs=3) as wT_p, \
             tc.tile_pool(name="ps_w", bufs=3, space="PSUM") as psw_p, \
             tc.tile_pool(name="ps_g1", bufs=4, space="PSUM") as psg1_p:
            for ot in range(NOT if "2" in stages else 0):
                wt = w_tile_T(wa, ot, wnat_p, wT_p, psw_p)
                ps = psg1_p.tile([128, TOK], F32)
                for kt in range(KT):
                    nc.tensor.matmul(ps[:], wt[:, kt, :], xT[:, kt, :],
                                     start=(kt == 0), stop=(kt == KT - 1))
                scale = 0.125 if 32 <= ot < 64 else 1.0
                nc.scalar.activation(qkvT[:, ot, :], ps[:], AFT.Identity,
                                     bias=bias_a[:, ot:ot + 1], scale=scale)

        # ---------- stage 3: attention ----------
        yT = None
        if "3" in stages or "4" in stages:
            yT = xy_p.tile([128, KT, TOK], BF16, tag="xy", name="yT")
            if "3" not in stages:
                nc.vector.memset(yT[:], 0.0)

        with tc.tile_pool(name="qwin", bufs=2) as qwin_p, \
             tc.tile_pool(name="kwin", bufs=2) as kwin_p, \
             tc.tile_pool(name="attbuf", bufs=2) as att_p, \
             tc.tile_pool(name="vbuf", bufs=2) as vb_p, \
             tc.tile_pool(name="ynorm", bufs=2) as yn_p, \
             tc.tile_pool(name="ps_att", bufs=2, space="PSUM") as psa_p, \
             tc.tile_pool(name="ps_v", bufs=2, space="PSUM") as psv_p, \
             tc.tile_pool(name="ps_y", bufs=2, space="PSUM") as psy_p, \
             tc.tile_pool(name="ps_yT", bufs=2, space="PSUM") as psyt_p:
            for w in range(TOK // TW if "3" in stages else 0):
                t0 = w * TW
                # h_pi layout: even heads at 0..31, odd heads at 32..63
                qw = qwin_p.tile([64, H, TW], BF16)
                kw = kwin_p.tile([64, H, TW], BF16)
                for par in range(2):
                    nc.sync.dma_start(
                        qw[:, par * 32:(par + 1) * 32, :],
                        qkvT[par * 64:(par + 1) * 64, 0:32, t0:t0 + TW])
                    nc.sync.dma_start(
                        kw[:, par * 32:(par + 1) * 32, :],
                        qkvT[par * 64:(par + 1) * 64, 32:64, t0:t0 + TW])

                for g in range(TW // TB):
                    att_ps = psa_p.tile([64, TB, 64], F32)
                    v_ps = psv_p.tile([64, TB, 64], BF16)
                    y_ps = psy_p.tile([64, TB, 65], F32)
                    yt_ps = psyt_p.tile([128, TB, 32], BF16)
                    attf = att_p.tile([64, TB, 64], F32)
                    expT = att_p.tile([64, TB, 64], BF16)
                    vt = vb_p.tile([64, TB, 65], BF16)
                    yn = yn_p.tile([64, TB, 64], BF16)
                    rec = yn_p.tile([64, TB], F32)
                    for b in range(TB):
                        t = g * TB + b
                        nc.tensor.matmul(att_ps[:, b, :],
                                         kw[:, :, t], qw[:, :, t],
                                         start=True, stop=True)
                        vsrc = qkvT[:, 64:96, t0 + t]
                        nc.tensor.transpose(v_ps[0:32, b, :],
                                            vsrc[0:64], ident[0:64, 0:64])
                        nc.tensor.transpose(v_ps[32:64, b, :],
                                            vsrc[64:128], ident[64:128, 64:128])
                    nc.vector.tensor_add(attf[:], att_ps[:], maskT4[:])
                    nc.scalar.activation(expT[:], attf[:], AFT.Exp)
                    nc.gpsimd.memset(vt[:, :, 64], 1.0)
                    nc.vector.tensor_copy(vt[:, :, 0:64], v_ps[:])
                    for b in range(TB):
                        nc.tensor.matmul(y_ps[:, b, :],
                                         expT[:, b, :], vt[:, b, :],
                                         start=True, stop=True)
                    nc.vector.reciprocal(rec[:], y_ps[:, :, 64])
                    rc = rec[:]
                    rb = bass.AP(rc.tensor, rc.offset,
                                 [[TB, 64], [1, TB], [0, 64]])
                    nc.vector.tensor_mul(yn[:], y_ps[:, :, 0:64], rb)
                    for b in range(TB):
                        nc.tensor.transpose(yt_ps[0:64, b, :],
                                            yn[0:32, b, :], ident[0:32, 0:32])
                        nc.tensor.transpose(yt_ps[64:128, b, :],
                                            yn[32:64, b, :], ident[32:64, 32:64])
                    tb0 = t0 + g * TB
                    for par in range(2):
                        nc.vector.tensor_copy(
                            yT[par * 64:(par + 1) * 64, 0:32, tb0:tb0 + TB]
                            .rearrange("d c t -> d t c"),
                            yt_ps[par * 64:(par + 1) * 64])

        qkvT_p.release()

        # ---------- stage 4: GEMM2 + W_proj^T streaming ----------
        with tc.tile_pool(name="wnat2", bufs=3) as wnat2_p, \
             tc.tile_pool(name="wT2", bufs=8) as wT2_p, \
             tc.tile_pool(name="ps_w2", bufs=3, space="PSUM") as psw2_p, \
             tc.tile_pool(name="ps_g2", bufs=4, space="PSUM") as psg2_p, \
             tc.tile_pool(name="oev", bufs=4) as oev_p:
            for s in range(NSL2 if "4" in stages else 0):
                ni = OSL2 // 128
                wts = [w_tile_T(wp, s * ni + i, wnat2_p, wT2_p, psw2_p)
                       for i in range(ni)]
                for tt in range(TOK // 128):
                    ps = psg2_p.tile([128, OSL2], F32)
                    for i in range(ni):
                        for kt in range(KT):
                            nc.tensor.matmul(
                                ps[:, i * 128:(i + 1) * 128],
                                yT[:, kt, tt * 128:(tt + 1) * 128],
                                wts[i][:, kt, :],
                                start=(kt == 0), stop=False)
                        o0 = s * OSL2 + i * 128
                        nc.tensor.matmul(ps[:, i * 128:(i + 1) * 128],
                                         ones_row[:], bias_p[:, o0:o0 + 128],
                                         start=False, stop=True)
                    ev = oev_p.tile([128, OSL2], F32)
                    nc.vector.tensor_copy(ev[:], ps[:])
                    nc.sync.dma_start(
                        out[tt * 128:(tt + 1) * 128, s * OSL2:(s + 1) * OSL2],
                        ev[:])

        xy_p.release()


def make_const_inputs():
    ident = np.eye(128, dtype=np.float32)
    pi = np.concatenate([np.arange(0, 64, 2), np.arange(1, 64, 2)])
    maskT = np.where(pi[:, None] <= pi[None, :], 0.0, NEG).astype(np.float32)
    return ident, maskT


N_CORES = 8
_CACHE = {}


def _get_nc():
    if "nc" not in _CACHE:
        _CACHE["nc"] = build_nc()
    return _CACHE["nc"]


def kernel(x, W_atten, b_atten, W_proj, b_proj):
    from concourse.bass_utils import run_bass_kernel_spmd

    x = np.ascontiguousarray(np.asarray(x, dtype=np.float32))
    W_atten = np.ascontiguousarray(np.asarray(W_atten, dtype=np.float32))
    b_atten = np.ascontiguousarray(np.asarray(b_atten, dtype=np.float32))
    W_proj = np.ascontiguousarray(np.asarray(W_proj, dtype=np.float32))
    b_proj = np.ascontiguousarray(np.asarray(b_proj, dtype=np.float32))

    nc = _get_nc()
    ident, maskT = make_const_inputs()
    B, T, Cx = x.shape
    shard = B // N_CORES
    in_maps = []
    for i in range(N_CORES):
        in_maps.append({
            "x": x[i * shard:(i + 1) * shard].reshape(TOK, C),
            "W_atten": W_atten, "b_atten": b_atten,
            "W_proj": W_proj, "b_proj": b_proj,
            "idmat": ident, "maskT": maskT,
        })
    res = run_bass_kernel_spmd(nc, in_maps, list(range(N_CORES)))
    out = np.empty((B, T, Cx), dtype=np.float32)
    for i in range(N_CORES):
        out[i * shard:(i + 1) * shard] = res.results[i]["out"].reshape(shard, T, Cx)
    return out
